# revision 27
# baseline (speedup 1.0000x reference)
"""Trainium2 Bass kernel for DFMN prototypical-network loss (retrieval_knn).

Reference math (per pixel, 64-way episode):
    protos = prototypes[indices]                         # [64, C]
    logits = -(|q|^2 + |p|^2 - 2 q.p)                    # [N, 64]
    loss   = -mean(log_softmax(logits)[label])

Key identity: the per-pixel |q|^2 term is constant across classes, so it
cancels in log_softmax.  With G = q.p and z = 2G - pn (pn = |p|^2 per class):
    -logp[label] = LSE_c(z) - z[label]
    loss = [ sum_px LSE_c(z) - sum_img (2*rowsumG[label_b] - 196*pn[label_b]) ] / N

Device layout per core (64 images, data-parallel over 8 cores).  Work is
organized in 32 image pairs; each pair's G accumulates in its own PSUM bank
at partitions 0-63 (DoubleRow matmuls only support tile_position (0,0), so
the old PE column-tiling across partition halves is gone).  The exp step
re-stacks two pairs into the halves of one 128-partition SBUF block via a
partition-shifted ACT write, so the selector colsum still runs full-width:
  - G via TensorE:   G[64, 392] = protosT_k2.T @ Q_k2  (4 DoubleRow chunk
    pairs, fp8; prototype weights loaded once per chunk pair per 6-pair
    group and redundant LDWEIGHTS deduplicated)
  - exp via ScalarE: e = Exp(2/16*G' - pn)  (PSUM [64] -> SBUF half, bf16)
  - colsum via TensorE: s[32, 392] += selector.T @ e  (selector has a ones
    column per partition half; accumulated across all 16 blocks in one bank)
  - label rowsums via VectorE: r2[64, 64] = per-image free-dim reduce of G
  - final ScalarE Ln with accum_out -> per-row sums of log s
Host finishes: label gather from r2, pn terms, exact float64 mean.

Streaming dtype is fp8 e4m3: quarters the fp32 HBM traffic (~12.85 MB/core)
and runs the PE in DoubleRow perf mode — two 128-deep k-chunks folded into
one matmul at 2 contraction rows/cycle.  Prototypes are pre-scaled by 16 so
their fp8 quantization stays in the normal range; the exp activation folds
the 1/16 back in via its scale operand (2/16), and the host divides the
label rowsums by 16.  pn is computed from the quantized prototypes, so the
kernel computes the exact loss of a slightly perturbed (quantized) problem;
measured end-to-end rel err ~6e-4 vs the fp32 reference.
"""

import sys

for _p in ("/opt/trn_rl_repo",):
    if _p not in sys.path:
        sys.path.insert(0, _p)

import numpy as np

import concourse.bass as bass
import concourse.bacc as bacc
import concourse.tile as tile
from concourse import mybir
from concourse.bass_utils import run_bass_kernel_spmd

# Problem constants (nn_DFMNLoss: B=512, C=1024, 14x14 features, 64-way)
B = 512
C = 1024
F2 = 196          # 14 * 14 pixels per image
NWAY = 64
NCORES = 8
BPC = B // NCORES           # 64 images per core
NPAIR = BPC // 2            # 32 image pairs per core
NU = NPAIR // 2             # 16 e-blocks (2 pairs stacked per SBUF block)
PPG = 6                     # pairs per DMA group (PSUM: 7 g-banks + 1 s-bank)
KT = C // 128               # 8 contraction chunks of 128 channels
KT2 = KT // 2               # 4 DoubleRow chunk-pairs of 256 channels
PAIRCOLS = 2 * F2           # 392 pixel columns per pair
QCOLS = KT * PAIRCOLS       # 3136 fp8 elements per partition per pair
SCALE_P = 16.0              # prototype pre-scale before fp8 quantization

F32 = mybir.dt.float32
F16 = mybir.dt.float16
BF16 = mybir.dt.bfloat16
F8 = mybir.dt.float8e4
BF16_NP = mybir.dt.np(BF16)
F8_NP = mybir.dt.np(F8)

_CACHE = {}


def _dedup_ldweights(nc):
    """Drop InstLdweights that reload weights already resident in the PE
    array.  Tile emits one LDWEIGHTS per matmul; inside a group the same
    prototype chunk is loaded for every unit, and the loads serialize with
    the matmul stream on the PE.  Matmults here are non-self-loading
    (ldweights=False), so a dropped reload just reuses the array contents.
    Tracks state per 32-wide PE column group; only sync-free LDWs are
    dropped, and any dangling dependency names are remapped to the keeper.
    """
    removed = {}
    for blk in nc.m.functions[0].blocks:
        state = {}  # col_group -> (key, keeper_name)
        kept = []
        for inst in blk.instructions:
            if isinstance(inst, mybir.InstLdweights):
                a = inst.ins[0]
                tp = inst.tile_position or (0, 0)
                ts = inst.tile_size or (128, 128)
                key = (a.memref, a.offset, str(a.ap), str(a.dtype), tp, ts)
                cgs = range(tp[1] // 32, (tp[1] + ts[1] + 31) // 32)
                si = inst.sync_info
                clean = si is None or (not si.on_wait and not si.on_update)
                prev = [state.get(cg) for cg in cgs]
                if clean and all(p is not None and p[0] == key for p in prev):
                    removed[inst.name] = prev[0][1]
                    continue
                for cg in cgs:
                    state[cg] = (key, inst.name)
            kept.append(inst)
        blk.instructions[:] = kept
    if removed:
        for blk in nc.m.functions[0].blocks:
            for inst in blk.instructions:
                names = set(inst.sync_dependency_names()) | set(
                    inst.nosync_dependency_names()
                )
                if names & removed.keys():
                    inst.remap_dependency_names(
                        {k: v for k, v in removed.items() if k in names}
                    )
        for k in removed:
            nc.inst_map.pop(k, None)
    return len(removed)


def _build_nc():
    # Bacc (not raw Bass): its compile() pass splits multi-wait instructions
    # into event semaphores — walrus allows only one sync wait per instruction.
    nc = bacc.Bacc()
    # q is partition-major: row p holds every pair's channel-chunk data for
    # partition p, so a group DMA is 128 large contiguous descriptors (the
    # old pair-major layout needed pairs*128 small ones and its issue cost
    # delayed the stream start by ~8us).
    q = nc.dram_tensor("q", [128, NPAIR * QCOLS], F8, kind="ExternalInput")
    pT = nc.dram_tensor("pT", [128, KT * NWAY], F8, kind="ExternalInput")
    negpn2 = nc.dram_tensor("negpn2", [128, 1], F32, kind="ExternalInput")
    bsel2 = nc.dram_tensor("bsel2", [128, 2 * NPAIR - 1], BF16, kind="ExternalInput")
    rsum = nc.dram_tensor("rsum", [NWAY, BPC], F32, kind="ExternalOutput")
    lse = nc.dram_tensor("lse", [NPAIR, 1], F32, kind="ExternalOutput")

    with tile.TileContext(nc) as tc:
        with (
            tc.tile_pool(name="const", bufs=1) as cpool,
            tc.tile_pool(name="qin", bufs=6) as qpool,
            tc.tile_pool(name="qtail", bufs=1) as tpool,
            tc.tile_pool(name="acc", bufs=1) as apool,
            tc.tile_pool(name="gps", bufs=7, space="PSUM") as gpool,
            tc.tile_pool(name="sps", bufs=1, space="PSUM") as spool,
        ):
            # First query-group DMA is issued before the const DMAs (further
            # below) so the big HBM stream starts as early as possible; the
            # constants land while the first group is still in flight.
            gt0 = qpool.tile([128, PPG * QCOLS], F8, name="gt", tag="gt")
            nc.sync.dma_start(
                gt0[:, 0 : PPG * QCOLS], q[:, 0 : PPG * QCOLS]
            )

            p_sb = cpool.tile([128, KT * NWAY], F8)
            nc.sync.dma_start(p_sb[:], pT[:])
            npn_sb = cpool.tile([128, 1], F32)
            nc.sync.dma_start(npn_sb[:], negpn2[:])
            bsel_sb = cpool.tile([128, 2 * NPAIR - 1], BF16)
            nc.sync.dma_start(bsel_sb[:], bsel2[:])

            r_sb = apool.tile([NWAY, BPC], F32)
            lse_sb = apool.tile([NPAIR, 1], F32)
            ltmp = apool.tile([NPAIR, PAIRCOLS], F32)
            e_all = apool.tile([128, NU * PAIRCOLS], BF16)
            s_ps = spool.tile([NPAIR, PAIRCOLS], F32)

            # ACT warmup: absorb the npn DMA wait, the const-AP init wait and
            # the exp/ln table loads outside the hot loop.  Ends on Exp so the
            # Exp table is resident when the hot loop starts (a mid-loop
            # ACT_TABLE_LOAD costs ~1.3us).
            warm_a = cpool.tile([128, 1], F32)
            warm_b = cpool.tile([128, 1], F32)
            nc.scalar.activation(
                warm_b[:], npn_sb[:], mybir.ActivationFunctionType.Exp
            )
            nc.scalar.activation(
                warm_a[:], warm_b[:], mybir.ActivationFunctionType.Ln
            )
            nc.scalar.activation(
                warm_b[:], warm_a[:], mybir.ActivationFunctionType.Exp
            )

            def sel_matmul(u):
                # s_ps[2u, :]   += colsum over partitions 0..63  of e(u)
                # s_ps[2u+1, :] += colsum over partitions 64..127 of e(u)
                nc.tensor.matmul(
                    s_ps[:],
                    bsel_sb[:, NPAIR - 1 - 2 * u : 2 * NPAIR - 1 - 2 * u],
                    e_all[:, u * PAIRCOLS : (u + 1) * PAIRCOLS],
                    start=(u == 0),
                    stop=(u == NU - 1),
                    skip_group_check=True,
                )

            groups = [
                list(range(g, min(g + PPG, NPAIR)))
                for g in range(0, NPAIR, PPG)
            ]
            # blocks (pairs stacked 2-per-e-block) fully produced by group gi
            blocks_of = [
                [u for u in range(NU) if 2 * u in pairs and 2 * u + 1 in pairs]
                for pairs in groups
            ]
            for gi, pairs in enumerate(groups):
                gp = len(pairs)
                p0 = pairs[0]
                # One big DMA per group: amortizes DMA fixed costs and keeps
                # the scheduler from serializing pairs (weight reuse).
                tail = gi == len(groups) - 1
                if gi == 0:
                    gt = gt0
                elif tail:
                    # Last (2-pair) group: split its DMA into two half-k
                    # tiles so the k2<2 matmuls overlap the second half's
                    # arrival — this group's compute is the serial tail after
                    # the stream ends.
                    half = (KT2 // 2) * 2 * PAIRCOLS   # 1568 cols per pair
                    ha = tpool.tile([128, gp * half], F8, name="ha", tag="ha")
                    hb = tpool.tile([128, gp * half], F8, name="hb", tag="hb")
                    for jloc, p in enumerate(pairs):
                        nc.sync.dma_start(
                            ha[:, jloc * half : (jloc + 1) * half],
                            q[:, p * QCOLS : p * QCOLS + half],
                        )
                    for jloc, p in enumerate(pairs):
                        nc.sync.dma_start(
                            hb[:, jloc * half : (jloc + 1) * half],
                            q[:, p * QCOLS + half : (p + 1) * QCOLS],
                        )
                else:
                    gt = qpool.tile(
                        [128, PPG * QCOLS], F8, name="gt", tag="gt"
                    )
                    nc.sync.dma_start(
                        gt[:, 0 : gp * QCOLS],
                        q[:, p0 * QCOLS : (p0 + gp) * QCOLS],
                    )
                gps = {
                    p: gpool.tile([NWAY, PAIRCOLS], F32, name="gps", tag="gps")
                    for p in pairs
                }
                for k2 in range(KT2):
                    # DoubleRow: fold chunk pair (2k2, 2k2+1) into one matmul;
                    # weights [128, 2, 64] and moving data [128, 2, 392] have
                    # the two k-subtiles adjacent in the free dim (the packed
                    # layout already stores k-chunks contiguously).  Only
                    # tile_position (0,0) is ISA-legal in DoubleRow mode.
                    wk = p_sb[:, k2 * 128 : (k2 + 1) * 128].rearrange(
                        "p (two m) -> p two m", two=2
                    )
                    for jloc, p in enumerate(pairs):
                        if tail:
                            src = ha if k2 < KT2 // 2 else hb
                            kk = k2 % (KT2 // 2)
                            ca = jloc * half + kk * 2 * PAIRCOLS
                        else:
                            src = gt
                            ca = jloc * QCOLS + k2 * 2 * PAIRCOLS
                        nc.tensor.matmul(
                            gps[p][:],
                            wk,
                            src[:, ca : ca + 2 * PAIRCOLS].rearrange(
                                "p (two c) -> p two c", two=2
                            ),
                            tile_position=(0, 0),
                            perf_mode=mybir.MatmulPerfMode.DoubleRow,
                            start=(k2 == 0),
                            stop=(k2 == KT2 - 1),
                            skip_group_check=True,
                        )
                # Selector matmuls lag one group so the PE never stalls on
                # the ACT exp (exp(g-1) ran during this group's matmuls).
                if gi > 0:
                    for u in blocks_of[gi - 1]:
                        sel_matmul(u)
                for p in pairs:
                    # Partition-shifted ACT write stacks pair 2u (partitions
                    # 0-63) and pair 2u+1 (64-127) into e-block u so the
                    # selector colsum runs on full 128-partition tiles.
                    hbase = NWAY * (p % 2)
                    u = p // 2
                    nc.scalar.activation(
                        e_all[hbase : hbase + NWAY,
                              u * PAIRCOLS : (u + 1) * PAIRCOLS],
                        gps[p][:],
                        mybir.ActivationFunctionType.Exp,
                        bias=npn_sb[0:NWAY, :],
                        scale=2.0 / SCALE_P,
                    )
                    # Single 3D reduce: [64, 2, 196] -X-> [64, 2] gives both
                    # images' rowsums in one DVE instruction.
                    nc.vector.reduce_sum(
                        r_sb[:, 2 * p : 2 * p + 2],
                        gps[p][:].rearrange("q (i f) -> q i f", i=2),
                        axis=mybir.AxisListType.X,
                    )
            # Bulk of the label-rowsum output can ship while the tail group
            # finishes (columns of pairs 0..29 are final before the tail).
            nc.sync.dma_start(
                rsum[:, 0 : 2 * (NPAIR - 2)], r_sb[:, 0 : 2 * (NPAIR - 2)]
            )
            for u in blocks_of[-1]:
                sel_matmul(u)

            nc.scalar.activation(
                ltmp[:],
                s_ps[:],
                mybir.ActivationFunctionType.Ln,
                accum_out=lse_sb[:],
            )
            nc.sync.dma_start(
                rsum[:, 2 * (NPAIR - 2) : BPC], r_sb[:, 2 * (NPAIR - 2) : BPC]
            )
            nc.sync.dma_start(lse[:], lse_sb[:])

    n = _dedup_ldweights(nc)
    if n < 64:
        print(f"[kernel] warning: ldweights dedup removed only {n}", flush=True)
    nc.compile()
    return nc


def _get_nc():
    if "nc" not in _CACHE:
        _CACHE["nc"] = _build_nc()
    return _CACHE["nc"]


def _pack_core_q(qc32):
    # [64, C, F2] -> [p, pair, k, i, f] -> [128, NPAIR*QCOLS] fp8 e4m3
    # (partition-major so each group DMA is a plain 2D column slice)
    qc = qc32.reshape(NPAIR, 2, KT, 128, F2).transpose(3, 0, 2, 1, 4)
    return np.ascontiguousarray(qc).astype(F8_NP).reshape(128, NPAIR * QCOLS)


def _prepare(query_features, labels, prototypes, indices):
    """Returns (in_maps, labels_i64, pn32)."""
    qf = np.asarray(query_features, dtype=np.float32).reshape(B, C, F2)
    labels = np.asarray(labels).astype(np.int64)
    protos = np.asarray(prototypes, dtype=np.float32)
    idx = np.asarray(indices).astype(np.int64)

    pg = protos[idx]                                     # [64, C] fp32
    # Quantize the scaled prototypes first; pn comes from the quantized
    # values so the kernel computes the exact loss of the perturbed problem.
    pq8 = (pg * SCALE_P).astype(F8_NP)                   # [64, C] fp8
    pq64 = pq8.astype(np.float64) / SCALE_P
    pn32 = np.sum(pq64 ** 2, axis=1).astype(np.float32)
    negpn2_np = np.ascontiguousarray(
        np.concatenate([-pn32, -pn32]).reshape(128, 1)
    )
    pT_pack = np.ascontiguousarray(
        pq8.T.reshape(KT, 128, NWAY).transpose(1, 0, 2)
    ).reshape(128, KT * NWAY)
    bsel2_np = np.zeros((128, 2 * NPAIR - 1), dtype=BF16_NP)
    bsel2_np[0:NWAY, NPAIR - 1] = 1
    bsel2_np[NWAY:128, NPAIR] = 1

    in_maps = [
        {
            "q": _pack_core_q(qf[c * BPC : (c + 1) * BPC]),
            "pT": pT_pack,
            "negpn2": negpn2_np,
            "bsel2": bsel2_np,
        }
        for c in range(NCORES)
    ]
    return in_maps, labels, pn32


def kernel(query_features, labels, prototypes, indices, n_way):
    import time as _time

    t0 = _time.time()
    nc = _get_nc()
    t1 = _time.time()
    in_maps, labels, pn32 = _prepare(query_features, labels, prototypes, indices)
    t2 = _time.time()
    results = run_bass_kernel_spmd(nc, in_maps, list(range(NCORES))).results
    t3 = _time.time()
    print(
        f"[kernel] build={t1 - t0:.1f}s pack={t2 - t1:.1f}s run={t3 - t2:.1f}s",
        flush=True,
    )

    # Host-side finish: r2[64, 64] holds per-image rowsums of G' (=16*G);
    # image local index l is column l, class is the row.
    pn64 = pn32.astype(np.float64)
    larr = np.arange(BPC)
    total_lse = 0.0
    label_term = 0.0
    for c in range(NCORES):
        total_lse += float(results[c]["lse"].astype(np.float64).sum())
        r2 = results[c]["rsum"].astype(np.float64)       # [64, 64]
        lab = labels[c * BPC : (c + 1) * BPC]
        label_term += float(
            np.sum((2.0 / SCALE_P) * r2[lab, larr] - F2 * pn64[lab])
        )
    loss = (total_lse - label_term) / (B * F2)
    return np.asarray(loss, dtype=np.float32)

